# revision 21
# baseline (speedup 1.0000x reference)
"""2-layer GCN (GCNConv -> ReLU -> GCNConv) on 8 Trainium2 NeuronCores.

Math:  out = Ahat @ relu(Ahat @ X @ W1 + b1) @ W2 + b2,
       Ahat = D^-1/2 (A + I) D^-1/2  (in-degree from dst, self-loops added).

Strategy (all hardcoded for N=100000, E=3200000, 512->16->40, 8 cores):
  * Factor the symmetric norm: pre-scale table rows by dinv, post-scale
    aggregated rows by dinv, self-loop handled algebraically (+table[own row]).
  * Commute W2 past aggregation: both layers aggregate 16-wide tables.
  * Nodes sharded 8-way, degree-profile interleaved so every core has an
    identical compile-time slot schedule (SPMD: one instruction stream).
  * Tables are bf16. Cross-core exchange is a COMPACT [12544,16] bf16
    AllGather (0.4MB contribution) -- the 256B-stride gather table is then
    re-spread locally into DRAM via contiguous staged DMA (table rows are
    numbered p-major: row = core*12544 + p*98 + c, so both the compact
    write and the padded spread are contiguous per partition).
  * Per layer: matmul/epilogue -> compact shard -> AllGather -> spread ->
    dma_gather (SWDGE, 4 queues, int16 indices mid-anchored per half-table
    segment, 32B payload on a 256B-stride table) into per-node slot grids
    -> strided DVE reduce -> epilogue.
"""

import sys

sys.path.insert(0, "/opt/trn_rl_repo")

import inspect
import textwrap

import numpy as np

import concourse.bacc as bacc
import concourse.bass as bass
import concourse.mybir as mybir
import concourse.tile as tile
from concourse._compat import cdiv
from concourse.bass_utils import run_bass_kernel_spmd
from concourse.masks import make_identity

F32 = mybir.dt.float32
BF16 = mybir.dt.bfloat16
I16 = mybir.dt.int16

N_CORES = 8
P = 128


# dma_gather with the elem_size%256 assert relaxed: the SWDGE ucode supports an
# arbitrary payload per index; only the row STRIDE must be a multiple of 256B.
def _make_dma_gather_raw():
    s = textwrap.dedent(inspect.getsource(bass.BassGpSimd.dma_gather))
    old = """    assert (
        elem_size_bytes > 0 and elem_size_bytes % 256 == 0
    )  # transpose restriction"""
    new = """    assert elem_size_bytes > 0
    if transpose:
        assert elem_size_bytes % 256 == 0"""
    assert old in s
    s = s.replace(old, new)
    g = dict(bass.__dict__)
    exec(compile(s, "<dma_gather_raw>", "exec"), g)
    return g["dma_gather"]


DMA_GATHER_RAW = _make_dma_gather_raw()


class Config:
    def __init__(self, n_nodes, f_in, hidden, f_out, percore, lmax_call=24):
        assert percore % P == 0
        self.n_nodes = n_nodes
        self.f_in = f_in
        self.hidden = hidden
        self.f_out = f_out
        self.percore = percore
        self.tiles = percore // P
        self.nrows = percore * N_CORES
        self.row_pad = 128          # table row stride in bf16 elements (256B)
        self.lmax_call = lmax_call  # max slot-depth per dma_gather call
        # segments: table rows [0, nrows/2) and [nrows/2, nrows); each must
        # span <= 65536 rows for int16 mid-anchored indices.
        half = self.nrows // 2
        assert half <= 65536
        self.seg_bounds = [(0, half), (half, self.nrows)]
        if half <= 32768:
            self.seg_anchor = [0, half]          # sim-friendly: no negative idxs
        else:
            self.seg_anchor = [half // 2, half + half // 2]
        # pad slots point at a guaranteed-zero table row with a POSITIVE
        # relative offset (trailing negative idxs would be trimmed by ucode):
        # row (half-1) / (nrows-1) is core 3/7's (p=127, c=tiles-1) node,
        # i.e. within-core sorted position 12543 >= 12500: a zero pad node.
        self.pad_row = [half - 1, self.nrows - 1]
        for g in (0, 1):
            rel = self.pad_row[g] - self.seg_anchor[g]
            assert 0 < rel <= 32767, rel
        lo0 = 0 - self.seg_anchor[0]
        hi0 = self.seg_bounds[0][1] - 1 - self.seg_anchor[0]
        lo1 = self.seg_bounds[1][0] - self.seg_anchor[1]
        hi1 = self.seg_bounds[1][1] - 1 - self.seg_anchor[1]
        assert lo0 >= -32768 and hi0 <= 32767
        assert lo1 >= -32768 and hi1 <= 32767


def host_prep(x, edge_index, cfg: Config, interleave=True):
    """Graph partitioning: relabel nodes, build per-core slot grids + schedule."""
    n = cfg.n_nodes
    T = cfg.tiles
    src_o = np.asarray(edge_index[0], dtype=np.int64)
    dst_o = np.asarray(edge_index[1], dtype=np.int64)
    deg = np.bincount(dst_o, minlength=n).astype(np.int64)  # in-degree, no self loop

    # pass 1: global degree sort -> core assignment (rank % 8) so all cores
    # see nearly identical degree profiles.
    rank_of = np.empty(n, dtype=np.int64)
    order = np.argsort(deg, kind="stable")
    rank_of[order] = np.arange(n)
    if interleave:
        core_of = rank_of % N_CORES
    else:
        core_of = rank_of // (n // N_CORES)

    # segment of a node depends only on its core (cores 0..3 -> seg0)
    seg_of_core = (np.arange(N_CORES) >= N_CORES // 2).astype(np.int64)
    dseg0 = np.bincount(dst_o[seg_of_core[core_of[src_o]] == 0], minlength=n)

    # pass 2: within each core order nodes by (deg, dseg0) for tight grids.
    # row_of: i-order rows (tile c = i//128, partition p = i%128) used for
    # x/deg packing and the output. trow_of: p-major table rows
    # (core*percore + p*T + c) used for gather-table addressing.
    row_of = np.empty(n, dtype=np.int64)
    trow_of = np.empty(n, dtype=np.int64)
    import os as _os
    SNAKE = int(_os.environ.get("SNAKE", "16"))  # tiles per d0-resort group
    for c in range(N_CORES):
        nodes = np.where(core_of == c)[0]
        o = np.lexsort((dseg0[nodes], deg[nodes]))
        nodes = nodes[o]
        if SNAKE > 1:
            B = P * SNAKE
            for s in range(0, len(nodes), B):
                blk = nodes[s:s + B]
                nodes[s:s + B] = blk[np.argsort(dseg0[blk], kind="stable")]
        i = np.arange(len(nodes))
        row_of[nodes] = c * cfg.percore + i
        trow_of[nodes] = c * cfg.percore + (i % P) * T + (i // P)

    src_r = trow_of[src_o]      # table rows (gather side)
    dst_r = row_of[dst_o]       # grid rows (dst side)

    # group edges by (dst core, dst local, segment of src)
    dst_core = dst_r // cfg.percore
    dst_local = dst_r % cfg.percore
    seg_src = (src_r >= cfg.seg_bounds[1][0]).astype(np.int64)

    # slot depth per (core, tile, seg) -> global schedule L[t][g]
    dst_tile = dst_local // P
    key = ((dst_core * T + dst_tile) * 2 + seg_src) * P + (dst_local % P)
    cnt = np.bincount(key, minlength=N_CORES * T * 2 * P).reshape(
        N_CORES, T, 2, P
    )
    Lmax = cnt.max(axis=(0, 3))  # [tiles, 2]

    # schedule: packed calls — chunks (one chunk = one slot-layer of one tile,
    # 128 idxs) are concatenated into calls of up to lmax_call chunks.
    import os as _os
    SUPER = int(_os.environ.get("SUPER", "1"))
    schedule = []  # (g, runs, off_w) with runs = [(tile, l_lo, l_hi), ...]
    off_w = 0
    for st in range(0, T, SUPER):
        for g in (0, 1):
            cur, cc = [], 0
            for t in range(st, min(st + SUPER, T)):
                L = int(Lmax[t, g])
                l = 0
                while l < L:
                    take = min(cfg.lmax_call - cc, L - l)
                    cur.append((t, l, l + take))
                    cc += take
                    l += take
                    if cc == cfg.lmax_call:
                        schedule.append((g, cur, off_w))
                        off_w += cc * P // 16
                        cur, cc = [], 0
            if cc:
                schedule.append((g, cur, off_w))
                off_w += cc * P // 16
    total_w = off_w

    # per-core idx arrays [128, total_w] int16
    idx_arrays = []
    order_e = np.lexsort((src_r, seg_src, dst_r))  # by dst, then seg, then src
    s_r = src_r[order_e]
    d_r = dst_r[order_e]
    g_r = seg_src[order_e]
    for c in range(N_CORES):
        m = (d_r // cfg.percore) == c
        s_c, d_c, g_c = s_r[m], d_r[m] % cfg.percore, g_r[m]
        # slot index within (node, seg): position among equal (d_c, g_c)
        grp = d_c * 2 + g_c
        slot = np.arange(len(grp)) - np.repeat(
            np.concatenate(([0], np.cumsum(np.bincount(grp, minlength=cfg.percore * 2))))[:-1],
            np.bincount(grp, minlength=cfg.percore * 2),
        )
        tiles_c = d_c // P
        p_c = d_c % P
        rel = s_c - np.where(g_c == 0, cfg.seg_anchor[0], cfg.seg_anchor[1])
        out = np.empty((128, total_w), dtype=np.int16)
        chunk_base = {}
        for g, runs, ow in schedule:
            pad_rel = cfg.pad_row[g] - cfg.seg_anchor[g]
            cc = 0
            for (t, l_lo, l_hi) in runs:
                for l in range(l_lo, l_hi):
                    chunk_base[(t, g, l)] = ow + cc * P // 16
                    cc += 1
            out[:, ow:ow + cc * P // 16] = pad_rel
        lmax_all = int(Lmax.max())
        cb = np.full((T, 2, lmax_all), -1, dtype=np.int64)
        for (t, g, l), wb in chunk_base.items():
            cb[t, g, l] = wb
        wb_arr = cb[tiles_c, g_c, slot]
        assert (wb_arr >= 0).all()
        w_pos = wb_arr + p_c // 16
        p_pos = p_c % 16
        vals = rel.astype(np.int16)
        for rep in range(8):
            out[p_pos + rep * 16, w_pos] = vals
        idx_arrays.append(out)

    deg_full = deg + 1  # self loop
    return {
        "row_of": row_of,
        "trow_of": trow_of,
        "core_of": core_of,
        "deg_full": deg_full,
        "schedule": schedule,
        "total_w": total_w,
        "idx_arrays": idx_arrays,
        "Lmax": Lmax,
    }


def build_bass(cfg: Config, schedule, total_w, phases=(1, 2, 3)):
    H, FO, FI = cfg.hidden, cfg.f_out, cfg.f_in
    RP = cfg.row_pad
    PC, T = cfg.percore, cfg.tiles
    KC = FI // P  # W1 contraction chunks

    import os as _os

    nc = bacc.Bacc(None, num_swdge_queues=4)
    xt = nc.dram_tensor("xt", [FI, PC], BF16, kind="ExternalInput")
    w1 = nc.dram_tensor("w1", [FI, H], BF16, kind="ExternalInput")
    w2 = nc.dram_tensor("w2", [H, FO], F32, kind="ExternalInput")
    b1t = nc.dram_tensor("b1t", [P, H], F32, kind="ExternalInput")
    b2t = nc.dram_tensor("b2t", [P, FO], F32, kind="ExternalInput")
    degt = nc.dram_tensor("degt", [PC], F32, kind="ExternalInput")
    maskt = nc.dram_tensor("maskt", [P, 1], F32, kind="ExternalInput")  # last tile rows
    idxs = nc.dram_tensor("idxs", [P, total_w], I16, kind="ExternalInput")
    out_d = nc.dram_tensor("out", [PC, FO], F32, kind="ExternalOutput")

    tab1_locH = [nc.dram_tensor(f"tab1_loc{h}", [PC // 2, H], BF16) for h in range(2)]
    tab2_locH = [nc.dram_tensor(f"tab2_loc{h}", [PC // 2, H], BF16) for h in range(2)]
    tab1_catH = [nc.dram_tensor(f"tab1_cat{h}", [cfg.nrows // 2, H], BF16, addr_space="Shared") for h in range(2)]
    tab2_catH = [nc.dram_tensor(f"tab2_cat{h}", [cfg.nrows // 2, H], BF16, addr_space="Shared") for h in range(2)]
    tab1 = nc.dram_tensor("tab1", [cfg.nrows, RP], BF16)
    tab2 = nc.dram_tensor("tab2", [cfg.nrows, RP], BF16)

    rg = [list(range(N_CORES))]
    CH = T // 2  # spread chunk: half a core-block (49 tiles)

    with tile.TileContext(nc) as tc:
        with (
            tc.tile_pool(name="persist", bufs=1) as pp,
            tc.tile_pool(name="xs", bufs=2) as xs_pool,
            tc.tile_pool(name="work", bufs=int(_os.environ.get("GBUFS", "8"))) as wp,
            tc.tile_pool(name="red", bufs=16) as rp,
            tc.tile_pool(name="spread", bufs=int(_os.environ.get("SBUFS", "2"))) as sp_pool,
            tc.tile_pool(name="ag", bufs=1) as ag_pool,
            tc.tile_pool(name="psum", bufs=2, space="PSUM") as psp,
            tc.tile_pool(name="psum1", bufs=2, space="PSUM") as psp1,
        ):
            # ---- persistent small tensors ----
            w1_t = pp.tile([P, KC * H], BF16)     # 4 chunks side by side
            nc.sync.dma_start(out=w1_t[:].rearrange("p (k h) -> p k h", k=KC),
                              in_=w1.ap().rearrange("(k p) h -> p k h", p=P))
            w2_t = pp.tile([H, FO], F32)
            nc.sync.dma_start(out=w2_t[:], in_=w2.ap())
            b1_t = pp.tile([P, H], F32)
            nc.sync.dma_start(out=b1_t[:], in_=b1t.ap())
            b2_t = pp.tile([P, FO], F32)
            nc.sync.dma_start(out=b2_t[:], in_=b2t.ap())
            mask_t = pp.tile([P, 1], F32)
            nc.sync.dma_start(out=mask_t[:], in_=maskt.ap())
            ident = pp.tile([P, P], F32)
            make_identity(nc, ident[:])
            deg_t = pp.tile([P, T], F32)
            nc.sync.dma_start(out=deg_t[:], in_=degt.ap().rearrange("(t p) -> p t", p=P))
            dinv_t = pp.tile([P, T], F32)
            nc.vector.reciprocal(out=dinv_t[:], in_=deg_t[:])
            nc.scalar.activation(out=dinv_t[:], in_=dinv_t[:],
                                 func=mybir.ActivationFunctionType.Sqrt)
            idx_all = pp.tile([P, total_w], I16)
            for lo in range(0, total_w, 8192):
                hi = min(total_w, lo + 8192)
                nc.scalar.dma_start(out=idx_all[:, lo:hi], in_=idxs.ap()[:, lo:hi])
            tab1_s = pp.tile([P, T * H], BF16)  # resident own shard (table1)
            tab2_s = pp.tile([P, T * H], BF16)

            # pre-zero the spread staging buffers once: content columns get
            # rewritten each chunk, pad columns must stay zero.
            n_sbufs = int(_os.environ.get("SBUFS", "2"))
            sp_tiles = []
            for i in range(n_sbufs):
                spt = sp_pool.tile([P, CH * RP], BF16, tag="sp")
                nc.vector.memset(spt[:], 0.0)
                sp_tiles.append(spt)

            # ---- phase 1: table1 = dinv * (X @ W1), fill own shard ----
            BLK = int(_os.environ.get("BLK", "1280"))
            xt_ap = xt.ap()
            for blk in range(0, PC, BLK) if "x" not in _os.environ.get("SKIP", "") else []:
                bw = min(BLK, PC - blk)
                chunks = []
                for k in range(KC):
                    cte = xs_pool.tile([P, BLK], BF16, tag=f"xt{k}")
                    nc.sync.dma_start(out=cte[:, :bw], in_=xt_ap[k * P:(k + 1) * P, blk:blk + bw])
                    chunks.append(cte)
                for ti in range(bw // P):
                    t = blk // P + ti
                    ps = psp1.tile([P, H], F32, space="PSUM", tag="ps1")
                    for k in range(KC):
                        nc.tensor.matmul(
                            out=ps[:],
                            lhsT=chunks[k][:, ti * P:(ti + 1) * P],
                            rhs=w1_t[:, k * H:(k + 1) * H],
                            start=(k == 0), stop=(k == KC - 1),
                        )
                    # epilogue on ACT: table1_tile = dinv * ps
                    nc.scalar.activation(
                        out=tab1_s[:, t * H:(t + 1) * H], in_=ps[:],
                        func=mybir.ActivationFunctionType.Copy,
                        scale=dinv_t[:, t:t + 1],
                    )
            if "x" in _os.environ.get("SKIP", ""):
                nc.vector.memset(tab1_s[:], 0.0)

            # compact shard -> DRAM (contiguous per partition), split in two
            # tile-range halves so AllGather(h0) can start while phase-1/epi1
            # still fills h1.
            cw1h = [
                nc.sync.dma_start(
                    out=tab1_locH[h].ap().rearrange("(p c) f -> p c f", p=P),
                    in_=tab1_s[:, h * CH * H:(h + 1) * CH * H].rearrange("p (c f) -> p c f", f=H),
                )
                for h in range(2)
            ]
            if 2 not in phases:
                zz = pp.tile([P, FO], F32)
                nc.vector.memset(zz[:], 0.0)
                for t in range(T):
                    nc.sync.dma_start(out=out_d.ap()[t * P:(t + 1) * P, :], in_=zz[:])

            from bass_rust import add_dep_helper as _adh

            def add_dep_helper(a, b, reason=""):
                if isinstance(a, bass.BassInstruction):
                    a = a.ins
                if isinstance(b, bass.BassInstruction):
                    b = b.ins
                _adh(a, b, reason=reason)

            def spread_half(cat, tab, ag_inst, h, tag):
                """One tile-range half of the AllGather output -> padded
                256B-stride table. Returns per-segment spread-write fences."""
                agt = ag_pool.tile([P, N_CORES * CH * H], BF16, tag=f"agt{tag}{h}")
                ld = nc.sync.dma_start(
                    out=agt[:].rearrange("p (k c f) -> p k c f", k=N_CORES, c=CH),
                    in_=cat.ap().rearrange("(k p c) f -> p k c f", k=N_CORES, p=P),
                )
                add_dep_helper(ld, ag_inst, reason="allgather -> compact load")
                fences = {0: [], 1: []}
                for k in range(N_CORES):
                    spt = sp_tiles[(k + h * N_CORES) % n_sbufs]
                    nc.vector.tensor_copy(
                        out=spt[:].rearrange("p (c w) -> p c w", w=RP)[:, :, :H],
                        in_=agt[:].rearrange("p (k c f) -> p k c f", k=N_CORES, c=CH)[:, k, :, :],
                    )
                    # rows k*PC + p*T + (h*CH + c): per partition contiguous
                    eng = nc.sync if k % 2 else nc.scalar
                    wr = eng.dma_start(
                        out=tab.ap().rearrange("(k p c) w -> p k c w", k=N_CORES, p=P)
                            [:, k, h * CH:(h + 1) * CH, :],
                        in_=spt[:].rearrange("p (c w) -> p c w", w=RP),
                    )
                    fences[0 if k < N_CORES // 2 else 1].append(wr)
                return fences

            def ag_spread(locH, catH, tab, cwh, tag):
                fences = {0: [], 1: []}
                for h in range(2):
                    ag = nc.gpsimd.collective_compute(
                        "AllGather", mybir.AluOpType.bypass,
                        ins=[locH[h].ap()], outs=[catH[h].ap()], replica_groups=rg,
                    )
                    add_dep_helper(ag, cwh[h], reason="compact write -> allgather")
                    f = spread_half(catH[h], tab, ag, h, tag)
                    fences[0] += f[0]
                    fences[1] += f[1]
                return fences

            # ---- all-gather table1 (compact) + spread ----
            fences1 = None
            if "g" not in _os.environ.get("SKIP", "") and 2 in phases:
                fences1 = ag_spread(tab1_locH, tab1_catH, tab1, cw1h, 1)

            chunks_of = {}   # t -> number of chunks expected
            for g, runs, ow in schedule:
                for (t, l_lo, l_hi) in runs:
                    chunks_of[t] = chunks_of.get(t, 0) + (l_hi - l_lo)

            def agg_layer(tab_full, layer, epilogue, fences=None):
                """Packed calls; per-tile strided reduces; epilogue(t, u) fires
                once a tile's chunks are all reduced."""
                own = tab1_s if layer == 1 else tab2_s
                parts = {t: [] for t in range(T)}
                done = {t: 0 for t in range(T)}

                def finish(t):
                    pl = parts[t]
                    u = rp.tile([P, H], F32, tag=f"u{layer}")
                    nc.vector.tensor_add(out=u[:], in0=pl[0][:], in1=pl[1][:] if len(pl) > 1 else own[:, t * H:(t + 1) * H])
                    for extra in pl[2:]:
                        nc.vector.tensor_add(out=u[:], in0=u[:], in1=extra[:])
                    if len(pl) > 1:
                        nc.vector.tensor_add(out=u[:], in0=u[:], in1=own[:, t * H:(t + 1) * H])
                    epilogue(t, u)

                for call_i, (g, runs, ow) in enumerate(schedule):
                    C = sum(l_hi - l_lo for (_, l_lo, l_hi) in runs)
                    ni = C * P
                    gt = wp.tile([P, C * H], BF16, tag="g")
                    anchor = cfg.seg_anchor[g]
                    gi = DMA_GATHER_RAW(
                        nc.gpsimd,
                        gt[:].rearrange("p (c h) -> p c h", c=C),
                        tab_full.ap()[anchor:, :H],
                        idx_all[:, ow:ow + C * P // 16],
                        ni, ni, H, elem_step=RP,
                        queue_num=call_i % 4,
                        single_packet=False,
                    )
                    for f in (fences[g] if fences else ()):
                        add_dep_helper(gi, f, reason="table spread -> gather")
                    off = 0
                    for (t, l_lo, l_hi) in runs:
                        n = l_hi - l_lo
                        red = rp.tile([P, H], F32, tag=f"red{layer}")
                        nc.vector.tensor_reduce(
                            out=red[:],
                            in_=gt[:, off * H:(off + n) * H].rearrange("p (l h) -> p h l", h=H),
                            op=mybir.AluOpType.add, axis=mybir.AxisListType.X,
                        )
                        parts[t].append(red)
                        done[t] += n
                        off += n
                        if done[t] == chunks_of[t]:
                            finish(t)

            # ---- phase 2: layer-1 aggregation + epilogue -> table2 ----
            def epi1(t, u):
                # v = u*dinv + b1  -> table2_tile = dinv * relu(v)
                nc.vector.tensor_scalar_mul(out=u[:], in0=u[:], scalar1=dinv_t[:, t:t + 1])
                nc.vector.tensor_add(out=u[:], in0=u[:], in1=b1_t[:])
                nc.scalar.activation(
                    out=tab2_s[:, t * H:(t + 1) * H], in_=u[:],
                    func=mybir.ActivationFunctionType.Relu,
                    scale=dinv_t[:, t:t + 1],
                )
                if t == T - 1:
                    nc.vector.tensor_scalar_mul(
                        out=tab2_s[:, t * H:(t + 1) * H],
                        in0=tab2_s[:, t * H:(t + 1) * H], scalar1=mask_t[:, :1],
                    )

            if 2 in phases:
                agg_layer(tab1, 1, epi1, fences1)
                cw2h = [
                    nc.sync.dma_start(
                        out=tab2_locH[h].ap().rearrange("(p c) f -> p c f", p=P),
                        in_=tab2_s[:, h * CH * H:(h + 1) * CH * H].rearrange("p (c f) -> p c f", f=H),
                    )
                    for h in range(2)
                ]

            # ---- all-gather table2 (compact, two halves) + spread ----
            fences2 = None
            if 2 in phases and 3 in phases:
                fences2 = ag_spread(tab2_locH, tab2_catH, tab2, cw2h, 2)

            # ---- phase 3: layer-2 aggregation + W2 + b2 ----
            def epi2(t, u):
                # transpose u -> [H, P], matmul with W2, scale by dinv, + b2
                ps_t = psp.tile([P, P], F32, space="PSUM", tag="pst")
                nc.tensor.transpose(out=ps_t[:H, :], in_=u[:], identity=ident[:])
                uT = rp.tile([H, P], F32, tag="uT")
                nc.vector.tensor_copy(out=uT[:], in_=ps_t[:H, :])
                ps_o = psp.tile([P, FO], F32, space="PSUM", tag="pso")
                nc.tensor.matmul(out=ps_o[:], lhsT=uT[:], rhs=w2_t[:], start=True, stop=True)
                ot = rp.tile([P, FO], F32, tag="ot")
                nc.vector.tensor_scalar_mul(out=ot[:], in0=ps_o[:], scalar1=dinv_t[:, t:t + 1])
                nc.vector.tensor_add(out=ot[:], in0=ot[:], in1=b2_t[:])
                nc.sync.dma_start(out=out_d.ap()[t * P:(t + 1) * P, :], in_=ot[:])

            if 3 in phases and 2 in phases:
                agg_layer(tab2, 2, epi2, fences2)
            elif 2 in phases:
                zz = pp.tile([P, FO], F32)
                nc.vector.memset(zz[:], 0.0)
                for t in range(T):
                    nc.sync.dma_start(out=out_d.ap()[t * P:(t + 1) * P, :], in_=zz[:])

    nc.finalize()
    return nc


_CACHE = {}


def _get_compiled(cfg_key, cfg, prep):
    if cfg_key not in _CACHE:
        nc = build_bass(cfg, prep["schedule"], prep["total_w"])
        _CACHE[cfg_key] = nc
    return _CACHE[cfg_key]


def run(x, edge_index, W1, b1, W2, b2, cfg: Config, prep=None, nc=None, time_iters=0):
    x = np.asarray(x, dtype=np.float32)
    W1 = np.asarray(W1, dtype=np.float32)
    b1 = np.asarray(b1, dtype=np.float32)
    W2 = np.asarray(W2, dtype=np.float32)
    b2 = np.asarray(b2, dtype=np.float32)
    if prep is None:
        prep = host_prep(x, edge_index, cfg)
    if nc is None:
        nc = _get_compiled(("main", cfg.n_nodes, cfg.percore), cfg, prep)

    in_maps = build_in_maps(x, W1, b1, W2, b2, cfg, prep)
    res = run_bass_kernel_spmd(nc, in_maps, core_ids=list(range(N_CORES)))
    out_rows = np.concatenate([res.results[c]["out"] for c in range(N_CORES)], axis=0)
    out = out_rows[prep["row_of"]]
    return out


def build_in_maps(x, W1, b1, W2, b2, cfg: Config, prep):
    import ml_dtypes
    row_of = prep["row_of"]
    deg_full = prep["deg_full"].astype(np.float32)
    x_rows = np.zeros((cfg.nrows, cfg.f_in), dtype=np.float32)
    x_rows[row_of] = x
    deg_rows = np.ones(cfg.nrows, dtype=np.float32)
    deg_rows[row_of] = deg_full
    b1t = np.tile(np.asarray(b1)[None, :], (P, 1)).astype(np.float32)
    b2t = np.tile(np.asarray(b2)[None, :], (P, 1)).astype(np.float32)
    real_pc = cfg.n_nodes // N_CORES
    mask_last = np.zeros((P, 1), dtype=np.float32)
    lastt = cfg.tiles - 1
    for p in range(P):
        mask_last[p, 0] = 1.0 if lastt * P + p < real_pc else 0.0
    in_maps = []
    for c in range(N_CORES):
        xs = x_rows[c * cfg.percore:(c + 1) * cfg.percore]
        in_maps.append({
            "xt": np.ascontiguousarray(xs.T).astype(ml_dtypes.bfloat16),
            "w1": np.asarray(W1, np.float32).astype(ml_dtypes.bfloat16),
            "w2": np.asarray(W2, np.float32),
            "b1t": b1t, "b2t": b2t,
            "degt": deg_rows[c * cfg.percore:(c + 1) * cfg.percore],
            "maskt": mask_last,
            "idxs": prep["idx_arrays"][c],
        })
    return in_maps


def kernel(x, edge_index, W1, b1, W2, b2):
    cfg = Config(100000, 512, 16, 40, percore=12544)
    return run(x, edge_index, W1, b1, W2, b2, cfg)


# revision 22
# speedup vs baseline: 1.1736x; 1.1736x over previous
"""2-layer GCN (GCNConv -> ReLU -> GCNConv) on 8 Trainium2 NeuronCores.

Math:  out = Ahat @ relu(Ahat @ X @ W1 + b1) @ W2 + b2,
       Ahat = D^-1/2 (A + I) D^-1/2  (in-degree from dst, self-loops added).

Strategy (all hardcoded for N=100000, E=3200000, 512->16->40, 8 cores):
  * Factor the symmetric norm: pre-scale table rows by dinv, post-scale
    aggregated rows by dinv, self-loop handled algebraically (+table[own row]).
  * Commute W2 past aggregation: both layers aggregate 16-wide tables.
  * Nodes sharded 8-way, degree-profile interleaved so every core has an
    identical compile-time slot schedule (SPMD: one instruction stream).
  * Tables are bf16. Cross-core exchange is a COMPACT [12544,16] bf16
    AllGather (0.4MB contribution) -- the 256B-stride gather table is then
    re-spread locally into DRAM via contiguous staged DMA (table rows are
    numbered p-major: row = core*12544 + p*98 + c, so both the compact
    write and the padded spread are contiguous per partition).
  * Per layer: matmul/epilogue -> compact shard -> AllGather -> spread ->
    dma_gather (SWDGE, 4 queues, int16 indices mid-anchored per half-table
    segment, 32B payload on a 256B-stride table) into per-node slot grids
    -> strided DVE reduce -> epilogue.
"""

import sys

sys.path.insert(0, "/opt/trn_rl_repo")

import inspect
import textwrap

import numpy as np

import concourse.bacc as bacc
import concourse.bass as bass
import concourse.mybir as mybir
import concourse.tile as tile
from concourse._compat import cdiv
from concourse.bass_utils import run_bass_kernel_spmd
from concourse.masks import make_identity

F32 = mybir.dt.float32
BF16 = mybir.dt.bfloat16
I16 = mybir.dt.int16

N_CORES = 8
P = 128


# dma_gather with the elem_size%256 assert relaxed: the SWDGE ucode supports an
# arbitrary payload per index; only the row STRIDE must be a multiple of 256B.
def _make_dma_gather_raw():
    s = textwrap.dedent(inspect.getsource(bass.BassGpSimd.dma_gather))
    old = """    assert (
        elem_size_bytes > 0 and elem_size_bytes % 256 == 0
    )  # transpose restriction"""
    new = """    assert elem_size_bytes > 0
    if transpose:
        assert elem_size_bytes % 256 == 0"""
    assert old in s
    s = s.replace(old, new)
    g = dict(bass.__dict__)
    exec(compile(s, "<dma_gather_raw>", "exec"), g)
    return g["dma_gather"]


DMA_GATHER_RAW = _make_dma_gather_raw()


class Config:
    def __init__(self, n_nodes, f_in, hidden, f_out, percore, lmax_call=24):
        assert percore % P == 0
        self.n_nodes = n_nodes
        self.f_in = f_in
        self.hidden = hidden
        self.f_out = f_out
        self.percore = percore
        self.tiles = percore // P
        self.nrows = percore * N_CORES
        self.row_pad = 128          # table row stride in bf16 elements (256B)
        self.lmax_call = lmax_call  # max slot-depth per dma_gather call
        # segments: table rows [0, nrows/2) and [nrows/2, nrows); each must
        # span <= 65536 rows for int16 mid-anchored indices.
        half = self.nrows // 2
        assert half <= 65536
        self.seg_bounds = [(0, half), (half, self.nrows)]
        if half <= 32768:
            self.seg_anchor = [0, half]          # sim-friendly: no negative idxs
        else:
            self.seg_anchor = [half // 2, half + half // 2]
        # pad slots point at a guaranteed-zero table row with a POSITIVE
        # relative offset (trailing negative idxs would be trimmed by ucode):
        # row (half-1) / (nrows-1) is core 3/7's (p=127, c=tiles-1) node,
        # i.e. within-core sorted position 12543 >= 12500: a zero pad node.
        self.pad_row = [half - 1, self.nrows - 1]
        for g in (0, 1):
            rel = self.pad_row[g] - self.seg_anchor[g]
            assert 0 < rel <= 32767, rel
        lo0 = 0 - self.seg_anchor[0]
        hi0 = self.seg_bounds[0][1] - 1 - self.seg_anchor[0]
        lo1 = self.seg_bounds[1][0] - self.seg_anchor[1]
        hi1 = self.seg_bounds[1][1] - 1 - self.seg_anchor[1]
        assert lo0 >= -32768 and hi0 <= 32767
        assert lo1 >= -32768 and hi1 <= 32767


def host_prep(x, edge_index, cfg: Config, interleave=True):
    """Graph partitioning: relabel nodes, build per-core slot grids + schedule."""
    n = cfg.n_nodes
    T = cfg.tiles
    src_o = np.asarray(edge_index[0], dtype=np.int64)
    dst_o = np.asarray(edge_index[1], dtype=np.int64)
    deg = np.bincount(dst_o, minlength=n).astype(np.int64)  # in-degree, no self loop

    # pass 1: global degree sort -> core assignment (rank % 8) so all cores
    # see nearly identical degree profiles.
    rank_of = np.empty(n, dtype=np.int64)
    order = np.argsort(deg, kind="stable")
    rank_of[order] = np.arange(n)
    if interleave:
        core_of = rank_of % N_CORES
    else:
        core_of = rank_of // (n // N_CORES)

    # segment of a node depends only on its core (cores 0..3 -> seg0)
    seg_of_core = (np.arange(N_CORES) >= N_CORES // 2).astype(np.int64)
    dseg0 = np.bincount(dst_o[seg_of_core[core_of[src_o]] == 0], minlength=n)

    # pass 2: within each core order nodes by (deg, dseg0) for tight grids.
    # row_of: i-order rows (tile c = i//128, partition p = i%128) used for
    # x/deg packing and the output. trow_of: p-major table rows
    # (core*percore + p*T + c) used for gather-table addressing.
    row_of = np.empty(n, dtype=np.int64)
    trow_of = np.empty(n, dtype=np.int64)
    import os as _os
    SNAKE = int(_os.environ.get("SNAKE", "16"))  # tiles per d0-resort group
    for c in range(N_CORES):
        nodes = np.where(core_of == c)[0]
        o = np.lexsort((dseg0[nodes], deg[nodes]))
        nodes = nodes[o]
        if SNAKE > 1:
            B = P * SNAKE
            for s in range(0, len(nodes), B):
                blk = nodes[s:s + B]
                nodes[s:s + B] = blk[np.argsort(dseg0[blk], kind="stable")]
        i = np.arange(len(nodes))
        row_of[nodes] = c * cfg.percore + i
        trow_of[nodes] = c * cfg.percore + (i % P) * T + (i // P)

    src_r = trow_of[src_o]      # table rows (gather side)
    dst_r = row_of[dst_o]       # grid rows (dst side)

    # group edges by (dst core, dst local, segment of src)
    dst_core = dst_r // cfg.percore
    dst_local = dst_r % cfg.percore
    seg_src = (src_r >= cfg.seg_bounds[1][0]).astype(np.int64)

    # slot depth per (core, tile, seg) -> global schedule L[t][g]
    dst_tile = dst_local // P
    key = ((dst_core * T + dst_tile) * 2 + seg_src) * P + (dst_local % P)
    cnt = np.bincount(key, minlength=N_CORES * T * 2 * P).reshape(
        N_CORES, T, 2, P
    )
    Lmax = cnt.max(axis=(0, 3))  # [tiles, 2]

    # schedule: packed calls — chunks (one chunk = one slot-layer of one tile,
    # 128 idxs) are concatenated into calls of up to lmax_call chunks.
    import os as _os
    SUPER = int(_os.environ.get("SUPER", "1"))
    schedule = []  # (g, runs, off_w) with runs = [(tile, l_lo, l_hi), ...]
    off_w = 0
    for st in range(0, T, SUPER):
        for g in (0, 1):
            cur, cc = [], 0
            for t in range(st, min(st + SUPER, T)):
                L = int(Lmax[t, g])
                l = 0
                while l < L:
                    take = min(cfg.lmax_call - cc, L - l)
                    cur.append((t, l, l + take))
                    cc += take
                    l += take
                    if cc == cfg.lmax_call:
                        schedule.append((g, cur, off_w))
                        off_w += cc * P // 16
                        cur, cc = [], 0
            if cc:
                schedule.append((g, cur, off_w))
                off_w += cc * P // 16
    total_w = off_w

    # per-core idx arrays [128, total_w] int16
    idx_arrays = []
    order_e = np.lexsort((src_r, seg_src, dst_r))  # by dst, then seg, then src
    s_r = src_r[order_e]
    d_r = dst_r[order_e]
    g_r = seg_src[order_e]
    for c in range(N_CORES):
        m = (d_r // cfg.percore) == c
        s_c, d_c, g_c = s_r[m], d_r[m] % cfg.percore, g_r[m]
        # slot index within (node, seg): position among equal (d_c, g_c)
        grp = d_c * 2 + g_c
        slot = np.arange(len(grp)) - np.repeat(
            np.concatenate(([0], np.cumsum(np.bincount(grp, minlength=cfg.percore * 2))))[:-1],
            np.bincount(grp, minlength=cfg.percore * 2),
        )
        tiles_c = d_c // P
        p_c = d_c % P
        rel = s_c - np.where(g_c == 0, cfg.seg_anchor[0], cfg.seg_anchor[1])
        out = np.empty((128, total_w), dtype=np.int16)
        pad_rels = {}
        for g in (0, 1):
            last_core = N_CORES // 2 - 1 if g == 0 else N_CORES - 1
            rows = last_core * cfg.percore + np.arange(84, 128) * T + (T - 1)
            rels = rows - cfg.seg_anchor[g]
            assert (rels > 0).all() and (rels <= 32767).all()
            pad_rels[g] = rels.astype(np.int16)
        chunk_base = {}
        for g, runs, ow in schedule:
            cc = 0
            for (t, l_lo, l_hi) in runs:
                for l in range(l_lo, l_hi):
                    chunk_base[(t, g, l)] = ow + cc * P // 16
                    cc += 1
            w = cc * P // 16
            blk = np.resize(pad_rels[g], (128, w))
            out[:, ow:ow + w] = blk
        lmax_all = int(Lmax.max())
        cb = np.full((T, 2, lmax_all), -1, dtype=np.int64)
        for (t, g, l), wb in chunk_base.items():
            cb[t, g, l] = wb
        wb_arr = cb[tiles_c, g_c, slot]
        assert (wb_arr >= 0).all()
        w_pos = wb_arr + p_c // 16
        p_pos = p_c % 16
        vals = rel.astype(np.int16)
        for rep in range(8):
            out[p_pos + rep * 16, w_pos] = vals
        idx_arrays.append(out)

    deg_full = deg + 1  # self loop
    return {
        "row_of": row_of,
        "trow_of": trow_of,
        "core_of": core_of,
        "deg_full": deg_full,
        "schedule": schedule,
        "total_w": total_w,
        "idx_arrays": idx_arrays,
        "Lmax": Lmax,
    }


def build_bass(cfg: Config, schedule, total_w, phases=(1, 2, 3)):
    H, FO, FI = cfg.hidden, cfg.f_out, cfg.f_in
    RP = cfg.row_pad
    PC, T = cfg.percore, cfg.tiles
    KC = FI // P  # W1 contraction chunks

    import os as _os

    nc = bacc.Bacc(None, num_swdge_queues=4)
    xt = nc.dram_tensor("xt", [FI, PC], BF16, kind="ExternalInput")
    w1 = nc.dram_tensor("w1", [FI, H], BF16, kind="ExternalInput")
    w2 = nc.dram_tensor("w2", [H, FO], F32, kind="ExternalInput")
    b1t = nc.dram_tensor("b1t", [P, H], F32, kind="ExternalInput")
    b2t = nc.dram_tensor("b2t", [P, FO], F32, kind="ExternalInput")
    degt = nc.dram_tensor("degt", [PC], F32, kind="ExternalInput")
    maskt = nc.dram_tensor("maskt", [P, 1], F32, kind="ExternalInput")  # last tile rows
    idxs = nc.dram_tensor("idxs", [P, total_w], I16, kind="ExternalInput")
    out_d = nc.dram_tensor("out", [PC, FO], F32, kind="ExternalOutput")

    tab1_locH = [nc.dram_tensor(f"tab1_loc{h}", [PC // 2, H], BF16) for h in range(2)]
    tab2_locH = [nc.dram_tensor(f"tab2_loc{h}", [PC // 2, H], BF16) for h in range(2)]
    tab1_catH = [nc.dram_tensor(f"tab1_cat{h}", [cfg.nrows // 2, H], BF16, addr_space="Shared") for h in range(2)]
    tab2_catH = [nc.dram_tensor(f"tab2_cat{h}", [cfg.nrows // 2, H], BF16, addr_space="Shared") for h in range(2)]
    tab1 = nc.dram_tensor("tab1", [cfg.nrows, RP], BF16)
    tab2 = nc.dram_tensor("tab2", [cfg.nrows, RP], BF16)

    rg = [list(range(N_CORES))]
    CH = T // 2  # spread chunk: half a core-block (49 tiles)

    with tile.TileContext(nc) as tc:
        with (
            tc.tile_pool(name="persist", bufs=1) as pp,
            tc.tile_pool(name="xs", bufs=2) as xs_pool,
            tc.tile_pool(name="work", bufs=int(_os.environ.get("GBUFS", "8"))) as wp,
            tc.tile_pool(name="red", bufs=16) as rp,
            tc.tile_pool(name="spread", bufs=int(_os.environ.get("SBUFS", "2"))) as sp_pool,
            tc.tile_pool(name="ag", bufs=1) as ag_pool,
            tc.tile_pool(name="psum", bufs=2, space="PSUM") as psp,
            tc.tile_pool(name="psum1", bufs=2, space="PSUM") as psp1,
        ):
            # ---- persistent small tensors ----
            w1_t = pp.tile([P, KC * H], BF16)     # 4 chunks side by side
            nc.sync.dma_start(out=w1_t[:].rearrange("p (k h) -> p k h", k=KC),
                              in_=w1.ap().rearrange("(k p) h -> p k h", p=P))
            w2_t = pp.tile([H, FO], F32)
            nc.sync.dma_start(out=w2_t[:], in_=w2.ap())
            b1_t = pp.tile([P, H], F32)
            nc.sync.dma_start(out=b1_t[:], in_=b1t.ap())
            b2_t = pp.tile([P, FO], F32)
            nc.sync.dma_start(out=b2_t[:], in_=b2t.ap())
            mask_t = pp.tile([P, 1], F32)
            nc.sync.dma_start(out=mask_t[:], in_=maskt.ap())
            ident = pp.tile([P, P], F32)
            make_identity(nc, ident[:])
            deg_t = pp.tile([P, T], F32)
            nc.sync.dma_start(out=deg_t[:], in_=degt.ap().rearrange("(t p) -> p t", p=P))
            dinv_t = pp.tile([P, T], F32)
            nc.vector.reciprocal(out=dinv_t[:], in_=deg_t[:])
            nc.scalar.activation(out=dinv_t[:], in_=dinv_t[:],
                                 func=mybir.ActivationFunctionType.Sqrt)
            idx_all = pp.tile([P, total_w], I16)
            for lo in range(0, total_w, 8192):
                hi = min(total_w, lo + 8192)
                nc.scalar.dma_start(out=idx_all[:, lo:hi], in_=idxs.ap()[:, lo:hi])
            tab1_s = pp.tile([P, T * H], BF16)  # resident own shard (table1)
            tab2_s = pp.tile([P, T * H], BF16)

            # pre-zero the spread staging buffers once: content columns get
            # rewritten each chunk, pad columns must stay zero.
            n_sbufs = int(_os.environ.get("SBUFS", "2"))
            sp_tiles = []
            for i in range(n_sbufs):
                spt = sp_pool.tile([P, CH * RP], BF16, tag="sp")
                nc.vector.memset(spt[:], 0.0)
                sp_tiles.append(spt)

            # ---- phase 1: table1 = dinv * (X @ W1), fill own shard ----
            BLK = int(_os.environ.get("BLK", "1280"))
            xt_ap = xt.ap()
            for blk in range(0, PC, BLK) if "x" not in _os.environ.get("SKIP", "") else []:
                bw = min(BLK, PC - blk)
                chunks = []
                for k in range(KC):
                    cte = xs_pool.tile([P, BLK], BF16, tag=f"xt{k}")
                    nc.sync.dma_start(out=cte[:, :bw], in_=xt_ap[k * P:(k + 1) * P, blk:blk + bw])
                    chunks.append(cte)
                for ti in range(bw // P):
                    t = blk // P + ti
                    ps = psp1.tile([P, H], F32, space="PSUM", tag="ps1")
                    for k in range(KC):
                        nc.tensor.matmul(
                            out=ps[:],
                            lhsT=chunks[k][:, ti * P:(ti + 1) * P],
                            rhs=w1_t[:, k * H:(k + 1) * H],
                            start=(k == 0), stop=(k == KC - 1),
                        )
                    # epilogue on ACT: table1_tile = dinv * ps
                    nc.scalar.activation(
                        out=tab1_s[:, t * H:(t + 1) * H], in_=ps[:],
                        func=mybir.ActivationFunctionType.Copy,
                        scale=dinv_t[:, t:t + 1],
                    )
            if "x" in _os.environ.get("SKIP", ""):
                nc.vector.memset(tab1_s[:], 0.0)

            # compact shard -> DRAM (contiguous per partition), split in two
            # tile-range halves so AllGather(h0) can start while phase-1/epi1
            # still fills h1.
            cw1h = [
                nc.sync.dma_start(
                    out=tab1_locH[h].ap().rearrange("(p c) f -> p c f", p=P),
                    in_=tab1_s[:, h * CH * H:(h + 1) * CH * H].rearrange("p (c f) -> p c f", f=H),
                )
                for h in range(2)
            ]
            if 2 not in phases:
                zz = pp.tile([P, FO], F32)
                nc.vector.memset(zz[:], 0.0)
                for t in range(T):
                    nc.sync.dma_start(out=out_d.ap()[t * P:(t + 1) * P, :], in_=zz[:])

            from bass_rust import add_dep_helper as _adh

            def add_dep_helper(a, b, reason=""):
                if isinstance(a, bass.BassInstruction):
                    a = a.ins
                if isinstance(b, bass.BassInstruction):
                    b = b.ins
                _adh(a, b, reason=reason)

            def spread_half(cat, tab, ag_inst, h, tag):
                """One tile-range half of the AllGather output -> padded
                256B-stride table. Returns per-segment spread-write fences."""
                agt = ag_pool.tile([P, N_CORES * CH * H], BF16, tag=f"agt{tag}{h}")
                ld = nc.sync.dma_start(
                    out=agt[:].rearrange("p (k c f) -> p k c f", k=N_CORES, c=CH),
                    in_=cat.ap().rearrange("(k p c) f -> p k c f", k=N_CORES, p=P),
                )
                add_dep_helper(ld, ag_inst, reason="allgather -> compact load")
                fences = {0: [], 1: []}
                for k in range(N_CORES):
                    spt = sp_tiles[(k + h * N_CORES) % n_sbufs]
                    nc.vector.tensor_copy(
                        out=spt[:].rearrange("p (c w) -> p c w", w=RP)[:, :, :H],
                        in_=agt[:].rearrange("p (k c f) -> p k c f", k=N_CORES, c=CH)[:, k, :, :],
                    )
                    # rows k*PC + p*T + (h*CH + c): per partition contiguous
                    eng = nc.sync if k % 2 else nc.scalar
                    wr = eng.dma_start(
                        out=tab.ap().rearrange("(k p c) w -> p k c w", k=N_CORES, p=P)
                            [:, k, h * CH:(h + 1) * CH, :],
                        in_=spt[:].rearrange("p (c w) -> p c w", w=RP),
                    )
                    fences[0 if k < N_CORES // 2 else 1].append(wr)
                return fences

            def ag_spread(locH, catH, tab, cwh, tag):
                fences = {0: [], 1: []}
                for h in range(2):
                    ag = nc.gpsimd.collective_compute(
                        "AllGather", mybir.AluOpType.bypass,
                        ins=[locH[h].ap()], outs=[catH[h].ap()], replica_groups=rg,
                    )
                    add_dep_helper(ag, cwh[h], reason="compact write -> allgather")
                    f = spread_half(catH[h], tab, ag, h, tag)
                    fences[0] += f[0]
                    fences[1] += f[1]
                return fences

            # ---- all-gather table1 (compact) + spread ----
            fences1 = None
            if "g" not in _os.environ.get("SKIP", "") and 2 in phases:
                fences1 = ag_spread(tab1_locH, tab1_catH, tab1, cw1h, 1)

            chunks_of = {}   # t -> number of chunks expected
            for g, runs, ow in schedule:
                for (t, l_lo, l_hi) in runs:
                    chunks_of[t] = chunks_of.get(t, 0) + (l_hi - l_lo)

            def agg_layer(tab_full, layer, epilogue, fences=None):
                """Packed calls; per-tile strided reduces; epilogue(t, u) fires
                once a tile's chunks are all reduced."""
                own = tab1_s if layer == 1 else tab2_s
                parts = {t: [] for t in range(T)}
                done = {t: 0 for t in range(T)}

                def finish(t):
                    pl = parts[t]
                    u = rp.tile([P, H], F32, tag=f"u{layer}")
                    nc.vector.tensor_add(out=u[:], in0=pl[0][:], in1=pl[1][:] if len(pl) > 1 else own[:, t * H:(t + 1) * H])
                    for extra in pl[2:]:
                        nc.vector.tensor_add(out=u[:], in0=u[:], in1=extra[:])
                    if len(pl) > 1:
                        nc.vector.tensor_add(out=u[:], in0=u[:], in1=own[:, t * H:(t + 1) * H])
                    epilogue(t, u)

                for call_i, (g, runs, ow) in enumerate(schedule):
                    C = sum(l_hi - l_lo for (_, l_lo, l_hi) in runs)
                    ni = C * P
                    gt = wp.tile([P, C * H], BF16, tag="g")
                    anchor = cfg.seg_anchor[g]
                    gi = DMA_GATHER_RAW(
                        nc.gpsimd,
                        gt[:].rearrange("p (c h) -> p c h", c=C),
                        tab_full.ap()[anchor:, :H],
                        idx_all[:, ow:ow + C * P // 16],
                        ni, ni, H, elem_step=RP,
                        queue_num=call_i % 4,
                        single_packet=False,
                    )
                    for f in (fences[g] if fences else ()):
                        add_dep_helper(gi, f, reason="table spread -> gather")
                    off = 0
                    for (t, l_lo, l_hi) in runs:
                        n = l_hi - l_lo
                        red = rp.tile([P, H], F32, tag=f"red{layer}")
                        nc.vector.tensor_reduce(
                            out=red[:],
                            in_=gt[:, off * H:(off + n) * H].rearrange("p (l h) -> p h l", h=H),
                            op=mybir.AluOpType.add, axis=mybir.AxisListType.X,
                        )
                        parts[t].append(red)
                        done[t] += n
                        off += n
                        if done[t] == chunks_of[t]:
                            finish(t)

            # ---- phase 2: layer-1 aggregation + epilogue -> table2 ----
            def epi1(t, u):
                # v = u*dinv + b1  -> table2_tile = dinv * relu(v)
                nc.vector.tensor_scalar_mul(out=u[:], in0=u[:], scalar1=dinv_t[:, t:t + 1])
                nc.vector.tensor_add(out=u[:], in0=u[:], in1=b1_t[:])
                nc.scalar.activation(
                    out=tab2_s[:, t * H:(t + 1) * H], in_=u[:],
                    func=mybir.ActivationFunctionType.Relu,
                    scale=dinv_t[:, t:t + 1],
                )
                if t == T - 1:
                    nc.vector.tensor_scalar_mul(
                        out=tab2_s[:, t * H:(t + 1) * H],
                        in0=tab2_s[:, t * H:(t + 1) * H], scalar1=mask_t[:, :1],
                    )

            if 2 in phases:
                agg_layer(tab1, 1, epi1, fences1)
                cw2h = [
                    nc.sync.dma_start(
                        out=tab2_locH[h].ap().rearrange("(p c) f -> p c f", p=P),
                        in_=tab2_s[:, h * CH * H:(h + 1) * CH * H].rearrange("p (c f) -> p c f", f=H),
                    )
                    for h in range(2)
                ]

            # ---- all-gather table2 (compact, two halves) + spread ----
            fences2 = None
            if 2 in phases and 3 in phases:
                fences2 = ag_spread(tab2_locH, tab2_catH, tab2, cw2h, 2)

            # ---- phase 3: layer-2 aggregation + W2 + b2 ----
            def epi2(t, u):
                # transpose u -> [H, P], matmul with W2, scale by dinv, + b2
                ps_t = psp.tile([P, P], F32, space="PSUM", tag="pst")
                nc.tensor.transpose(out=ps_t[:H, :], in_=u[:], identity=ident[:])
                uT = rp.tile([H, P], F32, tag="uT")
                nc.vector.tensor_copy(out=uT[:], in_=ps_t[:H, :])
                ps_o = psp.tile([P, FO], F32, space="PSUM", tag="pso")
                nc.tensor.matmul(out=ps_o[:], lhsT=uT[:], rhs=w2_t[:], start=True, stop=True)
                ot = rp.tile([P, FO], F32, tag="ot")
                nc.vector.tensor_scalar_mul(out=ot[:], in0=ps_o[:], scalar1=dinv_t[:, t:t + 1])
                nc.vector.tensor_add(out=ot[:], in0=ot[:], in1=b2_t[:])
                nc.sync.dma_start(out=out_d.ap()[t * P:(t + 1) * P, :], in_=ot[:])

            if 3 in phases and 2 in phases:
                agg_layer(tab2, 2, epi2, fences2)
            elif 2 in phases:
                zz = pp.tile([P, FO], F32)
                nc.vector.memset(zz[:], 0.0)
                for t in range(T):
                    nc.sync.dma_start(out=out_d.ap()[t * P:(t + 1) * P, :], in_=zz[:])

    nc.finalize()
    return nc


_CACHE = {}


def _get_compiled(cfg_key, cfg, prep):
    if cfg_key not in _CACHE:
        nc = build_bass(cfg, prep["schedule"], prep["total_w"])
        _CACHE[cfg_key] = nc
    return _CACHE[cfg_key]


def run(x, edge_index, W1, b1, W2, b2, cfg: Config, prep=None, nc=None, time_iters=0):
    x = np.asarray(x, dtype=np.float32)
    W1 = np.asarray(W1, dtype=np.float32)
    b1 = np.asarray(b1, dtype=np.float32)
    W2 = np.asarray(W2, dtype=np.float32)
    b2 = np.asarray(b2, dtype=np.float32)
    if prep is None:
        prep = host_prep(x, edge_index, cfg)
    if nc is None:
        nc = _get_compiled(("main", cfg.n_nodes, cfg.percore), cfg, prep)

    in_maps = build_in_maps(x, W1, b1, W2, b2, cfg, prep)
    res = run_bass_kernel_spmd(nc, in_maps, core_ids=list(range(N_CORES)))
    out_rows = np.concatenate([res.results[c]["out"] for c in range(N_CORES)], axis=0)
    out = out_rows[prep["row_of"]]
    return out


def build_in_maps(x, W1, b1, W2, b2, cfg: Config, prep):
    import ml_dtypes
    row_of = prep["row_of"]
    deg_full = prep["deg_full"].astype(np.float32)
    x_rows = np.zeros((cfg.nrows, cfg.f_in), dtype=np.float32)
    x_rows[row_of] = x
    deg_rows = np.ones(cfg.nrows, dtype=np.float32)
    deg_rows[row_of] = deg_full
    b1t = np.tile(np.asarray(b1)[None, :], (P, 1)).astype(np.float32)
    b2t = np.tile(np.asarray(b2)[None, :], (P, 1)).astype(np.float32)
    real_pc = cfg.n_nodes // N_CORES
    mask_last = np.zeros((P, 1), dtype=np.float32)
    lastt = cfg.tiles - 1
    for p in range(P):
        mask_last[p, 0] = 1.0 if lastt * P + p < real_pc else 0.0
    in_maps = []
    for c in range(N_CORES):
        xs = x_rows[c * cfg.percore:(c + 1) * cfg.percore]
        in_maps.append({
            "xt": np.ascontiguousarray(xs.T).astype(ml_dtypes.bfloat16),
            "w1": np.asarray(W1, np.float32).astype(ml_dtypes.bfloat16),
            "w2": np.asarray(W2, np.float32),
            "b1t": b1t, "b2t": b2t,
            "degt": deg_rows[c * cfg.percore:(c + 1) * cfg.percore],
            "maskt": mask_last,
            "idxs": prep["idx_arrays"][c],
        })
    return in_maps


def kernel(x, edge_index, W1, b1, W2, b2):
    cfg = Config(100000, 512, 16, 40, percore=12544)
    return run(x, edge_index, W1, b1, W2, b2, cfg)


# revision 23
# speedup vs baseline: 1.1933x; 1.0168x over previous
"""2-layer GCN (GCNConv -> ReLU -> GCNConv) on 8 Trainium2 NeuronCores.

Math:  out = Ahat @ relu(Ahat @ X @ W1 + b1) @ W2 + b2,
       Ahat = D^-1/2 (A + I) D^-1/2  (in-degree from dst, self-loops added).

Strategy (all hardcoded for N=100000, E=3200000, 512->16->40, 8 cores):
  * Factor the symmetric norm: pre-scale table rows by dinv, post-scale
    aggregated rows by dinv, self-loop handled algebraically (+table[own row]).
  * Commute W2 past aggregation: both layers aggregate 16-wide tables.
  * Nodes sharded 8-way, degree-profile interleaved so every core has an
    identical compile-time slot schedule (SPMD: one instruction stream).
  * Tables are bf16. Cross-core exchange is a COMPACT [12544,16] bf16
    AllGather (0.4MB contribution) -- the 256B-stride gather table is then
    re-spread locally into DRAM via contiguous staged DMA (table rows are
    numbered p-major: row = core*12544 + p*98 + c, so both the compact
    write and the padded spread are contiguous per partition).
  * Per layer: matmul/epilogue -> compact shard -> AllGather -> spread ->
    dma_gather (SWDGE, 4 queues, int16 indices mid-anchored per half-table
    segment, 32B payload on a 256B-stride table) into per-node slot grids
    -> strided DVE reduce -> epilogue.
"""

import sys

sys.path.insert(0, "/opt/trn_rl_repo")

import inspect
import textwrap

import numpy as np

import concourse.bacc as bacc
import concourse.bass as bass
import concourse.mybir as mybir
import concourse.tile as tile
from concourse._compat import cdiv
from concourse.bass_utils import run_bass_kernel_spmd
from concourse.masks import make_identity

F32 = mybir.dt.float32
BF16 = mybir.dt.bfloat16
I16 = mybir.dt.int16

N_CORES = 8
P = 128


# dma_gather with the elem_size%256 assert relaxed: the SWDGE ucode supports an
# arbitrary payload per index; only the row STRIDE must be a multiple of 256B.
def _make_dma_gather_raw():
    s = textwrap.dedent(inspect.getsource(bass.BassGpSimd.dma_gather))
    old = """    assert (
        elem_size_bytes > 0 and elem_size_bytes % 256 == 0
    )  # transpose restriction"""
    new = """    assert elem_size_bytes > 0
    if transpose:
        assert elem_size_bytes % 256 == 0"""
    assert old in s
    s = s.replace(old, new)
    g = dict(bass.__dict__)
    exec(compile(s, "<dma_gather_raw>", "exec"), g)
    return g["dma_gather"]


DMA_GATHER_RAW = _make_dma_gather_raw()


class Config:
    def __init__(self, n_nodes, f_in, hidden, f_out, percore, lmax_call=24):
        assert percore % P == 0
        self.n_nodes = n_nodes
        self.f_in = f_in
        self.hidden = hidden
        self.f_out = f_out
        self.percore = percore
        self.tiles = percore // P
        self.nrows = percore * N_CORES
        self.row_pad = 128          # table row stride in bf16 elements (256B)
        self.lmax_call = lmax_call  # max slot-depth per dma_gather call
        # segments: table rows [0, nrows/2) and [nrows/2, nrows); each must
        # span <= 65536 rows for int16 mid-anchored indices.
        half = self.nrows // 2
        assert half <= 65536
        self.seg_bounds = [(0, half), (half, self.nrows)]
        if half <= 32768:
            self.seg_anchor = [0, half]          # sim-friendly: no negative idxs
        else:
            self.seg_anchor = [half // 2, half + half // 2]
        # pad slots point at a guaranteed-zero table row with a POSITIVE
        # relative offset (trailing negative idxs would be trimmed by ucode):
        # row (half-1) / (nrows-1) is core 3/7's (p=127, c=tiles-1) node,
        # i.e. within-core sorted position 12543 >= 12500: a zero pad node.
        self.pad_row = [half - 1, self.nrows - 1]
        for g in (0, 1):
            rel = self.pad_row[g] - self.seg_anchor[g]
            assert 0 < rel <= 32767, rel
        lo0 = 0 - self.seg_anchor[0]
        hi0 = self.seg_bounds[0][1] - 1 - self.seg_anchor[0]
        lo1 = self.seg_bounds[1][0] - self.seg_anchor[1]
        hi1 = self.seg_bounds[1][1] - 1 - self.seg_anchor[1]
        assert lo0 >= -32768 and hi0 <= 32767
        assert lo1 >= -32768 and hi1 <= 32767


def host_prep(x, edge_index, cfg: Config, interleave=True):
    """Graph partitioning: relabel nodes, build per-core slot grids + schedule."""
    n = cfg.n_nodes
    T = cfg.tiles
    src_o = np.asarray(edge_index[0], dtype=np.int64)
    dst_o = np.asarray(edge_index[1], dtype=np.int64)
    deg = np.bincount(dst_o, minlength=n).astype(np.int64)  # in-degree, no self loop

    # pass 1: global degree sort -> core assignment (rank % 8) so all cores
    # see nearly identical degree profiles.
    rank_of = np.empty(n, dtype=np.int64)
    order = np.argsort(deg, kind="stable")
    rank_of[order] = np.arange(n)
    if interleave:
        core_of = rank_of % N_CORES
    else:
        core_of = rank_of // (n // N_CORES)

    # segment of a node depends only on its core (cores 0..3 -> seg0)
    seg_of_core = (np.arange(N_CORES) >= N_CORES // 2).astype(np.int64)
    dseg0 = np.bincount(dst_o[seg_of_core[core_of[src_o]] == 0], minlength=n)

    # pass 2: within each core order nodes by (deg, dseg0) for tight grids.
    # row_of: i-order rows (tile c = i//128, partition p = i%128) used for
    # x/deg packing and the output. trow_of: p-major table rows
    # (core*percore + p*T + c) used for gather-table addressing.
    row_of = np.empty(n, dtype=np.int64)
    trow_of = np.empty(n, dtype=np.int64)
    import os as _os
    SNAKE = int(_os.environ.get("SNAKE", "16"))  # tiles per d0-resort group
    for c in range(N_CORES):
        nodes = np.where(core_of == c)[0]
        o = np.lexsort((dseg0[nodes], deg[nodes]))
        nodes = nodes[o]
        if SNAKE > 1:
            B = P * SNAKE
            for s in range(0, len(nodes), B):
                blk = nodes[s:s + B]
                nodes[s:s + B] = blk[np.argsort(dseg0[blk], kind="stable")]
        i = np.arange(len(nodes))
        row_of[nodes] = c * cfg.percore + i
        trow_of[nodes] = c * cfg.percore + (i % P) * T + (i // P)

    src_r = trow_of[src_o]      # table rows (gather side)
    dst_r = row_of[dst_o]       # grid rows (dst side)

    # group edges by (dst core, dst local, segment of src)
    dst_core = dst_r // cfg.percore
    dst_local = dst_r % cfg.percore
    seg_src = (src_r >= cfg.seg_bounds[1][0]).astype(np.int64)

    # slot depth per (core, tile, seg) -> global schedule L[t][g]
    dst_tile = dst_local // P
    key = ((dst_core * T + dst_tile) * 2 + seg_src) * P + (dst_local % P)
    cnt = np.bincount(key, minlength=N_CORES * T * 2 * P).reshape(
        N_CORES, T, 2, P
    )
    Lmax = cnt.max(axis=(0, 3))  # [tiles, 2]

    # schedule: packed calls — chunks (one chunk = one slot-layer of one tile,
    # 128 idxs) are concatenated into calls of up to lmax_call chunks.
    import os as _os
    SUPER = int(_os.environ.get("SUPER", "1"))
    schedule = []  # (g, runs, off_w) with runs = [(tile, l_lo, l_hi), ...]
    off_w = 0
    for st in range(0, T, SUPER):
        for g in (0, 1):
            cur, cc = [], 0
            for t in range(st, min(st + SUPER, T)):
                L = int(Lmax[t, g])
                l = 0
                while l < L:
                    take = min(cfg.lmax_call - cc, L - l)
                    cur.append((t, l, l + take))
                    cc += take
                    l += take
                    if cc == cfg.lmax_call:
                        schedule.append((g, cur, off_w))
                        off_w += cc * P // 16
                        cur, cc = [], 0
            if cc:
                schedule.append((g, cur, off_w))
                off_w += cc * P // 16
    total_w = off_w

    # per-core idx arrays [128, total_w] int16
    idx_arrays = []
    order_e = np.lexsort((src_r, seg_src, dst_r))  # by dst, then seg, then src
    s_r = src_r[order_e]
    d_r = dst_r[order_e]
    g_r = seg_src[order_e]
    for c in range(N_CORES):
        m = (d_r // cfg.percore) == c
        s_c, d_c, g_c = s_r[m], d_r[m] % cfg.percore, g_r[m]
        # slot index within (node, seg): position among equal (d_c, g_c)
        grp = d_c * 2 + g_c
        slot = np.arange(len(grp)) - np.repeat(
            np.concatenate(([0], np.cumsum(np.bincount(grp, minlength=cfg.percore * 2))))[:-1],
            np.bincount(grp, minlength=cfg.percore * 2),
        )
        tiles_c = d_c // P
        p_c = d_c % P
        rel = s_c - np.where(g_c == 0, cfg.seg_anchor[0], cfg.seg_anchor[1])
        out = np.empty((128, total_w), dtype=np.int16)
        pad_rels = {}
        for g in (0, 1):
            # pad rows of the two last cores in each half: all have positive
            # rel offsets vs the mid-anchor, spreading pad reads over two
            # separate HBM regions.
            cores = (2, 3) if g == 0 else (6, 7)
            rows = np.concatenate([
                kk * cfg.percore + np.arange(84, 128) * T + (T - 1) for kk in cores
            ])
            rels = rows - cfg.seg_anchor[g]
            assert (rels > 0).all() and (rels <= 32767).all()
            pad_rels[g] = rels.astype(np.int16)
        chunk_base = {}
        for g, runs, ow in schedule:
            cc = 0
            for (t, l_lo, l_hi) in runs:
                for l in range(l_lo, l_hi):
                    chunk_base[(t, g, l)] = ow + cc * P // 16
                    cc += 1
            w = cc * P // 16
            blk = np.resize(pad_rels[g], (128, w))
            out[:, ow:ow + w] = blk
        lmax_all = int(Lmax.max())
        cb = np.full((T, 2, lmax_all), -1, dtype=np.int64)
        for (t, g, l), wb in chunk_base.items():
            cb[t, g, l] = wb
        wb_arr = cb[tiles_c, g_c, slot]
        assert (wb_arr >= 0).all()
        w_pos = wb_arr + p_c // 16
        p_pos = p_c % 16
        vals = rel.astype(np.int16)
        for rep in range(8):
            out[p_pos + rep * 16, w_pos] = vals
        idx_arrays.append(out)

    deg_full = deg + 1  # self loop
    return {
        "row_of": row_of,
        "trow_of": trow_of,
        "core_of": core_of,
        "deg_full": deg_full,
        "schedule": schedule,
        "total_w": total_w,
        "idx_arrays": idx_arrays,
        "Lmax": Lmax,
    }


def build_bass(cfg: Config, schedule, total_w, phases=(1, 2, 3)):
    H, FO, FI = cfg.hidden, cfg.f_out, cfg.f_in
    RP = cfg.row_pad
    PC, T = cfg.percore, cfg.tiles
    KC = FI // P  # W1 contraction chunks

    import os as _os

    nc = bacc.Bacc(None, num_swdge_queues=4)
    xt = nc.dram_tensor("xt", [FI, PC], BF16, kind="ExternalInput")
    w1 = nc.dram_tensor("w1", [FI, H], BF16, kind="ExternalInput")
    w2 = nc.dram_tensor("w2", [H, FO], F32, kind="ExternalInput")
    b1t = nc.dram_tensor("b1t", [P, H], F32, kind="ExternalInput")
    b2t = nc.dram_tensor("b2t", [P, FO], F32, kind="ExternalInput")
    degt = nc.dram_tensor("degt", [PC], F32, kind="ExternalInput")
    maskt = nc.dram_tensor("maskt", [P, 1], F32, kind="ExternalInput")  # last tile rows
    idxs = nc.dram_tensor("idxs", [P, total_w], I16, kind="ExternalInput")
    out_d = nc.dram_tensor("out", [PC, FO], F32, kind="ExternalOutput")

    tab1_locH = [nc.dram_tensor(f"tab1_loc{h}", [PC // 2, H], BF16) for h in range(2)]
    tab2_locH = [nc.dram_tensor(f"tab2_loc{h}", [PC // 2, H], BF16) for h in range(2)]
    tab1_catH = [nc.dram_tensor(f"tab1_cat{h}", [cfg.nrows // 2, H], BF16, addr_space="Shared") for h in range(2)]
    tab2_catH = [nc.dram_tensor(f"tab2_cat{h}", [cfg.nrows // 2, H], BF16, addr_space="Shared") for h in range(2)]
    tab1 = nc.dram_tensor("tab1", [cfg.nrows, RP], BF16)
    tab2 = nc.dram_tensor("tab2", [cfg.nrows, RP], BF16)

    rg = [list(range(N_CORES))]
    CH = T // 2  # spread chunk: half a core-block (49 tiles)

    with tile.TileContext(nc) as tc:
        with (
            tc.tile_pool(name="persist", bufs=1) as pp,
            tc.tile_pool(name="xs", bufs=2) as xs_pool,
            tc.tile_pool(name="work", bufs=int(_os.environ.get("GBUFS", "8"))) as wp,
            tc.tile_pool(name="red", bufs=16) as rp,
            tc.tile_pool(name="spread", bufs=int(_os.environ.get("SBUFS", "2"))) as sp_pool,
            tc.tile_pool(name="ag", bufs=1) as ag_pool,
            tc.tile_pool(name="psum", bufs=2, space="PSUM") as psp,
            tc.tile_pool(name="psum1", bufs=2, space="PSUM") as psp1,
        ):
            # ---- persistent small tensors ----
            w1_t = pp.tile([P, KC * H], BF16)     # 4 chunks side by side
            nc.sync.dma_start(out=w1_t[:].rearrange("p (k h) -> p k h", k=KC),
                              in_=w1.ap().rearrange("(k p) h -> p k h", p=P))
            w2_t = pp.tile([H, FO], F32)
            nc.sync.dma_start(out=w2_t[:], in_=w2.ap())
            b1_t = pp.tile([P, H], F32)
            nc.sync.dma_start(out=b1_t[:], in_=b1t.ap())
            b2_t = pp.tile([P, FO], F32)
            nc.sync.dma_start(out=b2_t[:], in_=b2t.ap())
            mask_t = pp.tile([P, 1], F32)
            nc.sync.dma_start(out=mask_t[:], in_=maskt.ap())
            ident = pp.tile([P, P], F32)
            make_identity(nc, ident[:])
            deg_t = pp.tile([P, T], F32)
            nc.sync.dma_start(out=deg_t[:], in_=degt.ap().rearrange("(t p) -> p t", p=P))
            dinv_t = pp.tile([P, T], F32)
            nc.vector.reciprocal(out=dinv_t[:], in_=deg_t[:])
            nc.scalar.activation(out=dinv_t[:], in_=dinv_t[:],
                                 func=mybir.ActivationFunctionType.Sqrt)
            idx_all = pp.tile([P, total_w], I16)
            for lo in range(0, total_w, 8192):
                hi = min(total_w, lo + 8192)
                nc.scalar.dma_start(out=idx_all[:, lo:hi], in_=idxs.ap()[:, lo:hi])
            tab1_s = pp.tile([P, T * H], BF16)  # resident own shard (table1)
            tab2_s = pp.tile([P, T * H], BF16)

            # pre-zero the spread staging buffers once: content columns get
            # rewritten each chunk, pad columns must stay zero.
            n_sbufs = int(_os.environ.get("SBUFS", "2"))
            sp_tiles = []
            for i in range(n_sbufs):
                spt = sp_pool.tile([P, CH * RP], BF16, tag="sp")
                nc.vector.memset(spt[:], 0.0)
                sp_tiles.append(spt)

            # ---- phase 1: table1 = dinv * (X @ W1), fill own shard ----
            BLK = int(_os.environ.get("BLK", "1280"))
            xt_ap = xt.ap()
            for blk in range(0, PC, BLK) if "x" not in _os.environ.get("SKIP", "") else []:
                bw = min(BLK, PC - blk)
                chunks = []
                for k in range(KC):
                    cte = xs_pool.tile([P, BLK], BF16, tag=f"xt{k}")
                    nc.sync.dma_start(out=cte[:, :bw], in_=xt_ap[k * P:(k + 1) * P, blk:blk + bw])
                    chunks.append(cte)
                for ti in range(bw // P):
                    t = blk // P + ti
                    ps = psp1.tile([P, H], F32, space="PSUM", tag="ps1")
                    for k in range(KC):
                        nc.tensor.matmul(
                            out=ps[:],
                            lhsT=chunks[k][:, ti * P:(ti + 1) * P],
                            rhs=w1_t[:, k * H:(k + 1) * H],
                            start=(k == 0), stop=(k == KC - 1),
                        )
                    # epilogue on ACT: table1_tile = dinv * ps
                    nc.scalar.activation(
                        out=tab1_s[:, t * H:(t + 1) * H], in_=ps[:],
                        func=mybir.ActivationFunctionType.Copy,
                        scale=dinv_t[:, t:t + 1],
                    )
            if "x" in _os.environ.get("SKIP", ""):
                nc.vector.memset(tab1_s[:], 0.0)

            # compact shard -> DRAM (contiguous per partition), split in two
            # tile-range halves so AllGather(h0) can start while phase-1/epi1
            # still fills h1.
            cw1h = [
                nc.sync.dma_start(
                    out=tab1_locH[h].ap().rearrange("(p c) f -> p c f", p=P),
                    in_=tab1_s[:, h * CH * H:(h + 1) * CH * H].rearrange("p (c f) -> p c f", f=H),
                )
                for h in range(2)
            ]
            if 2 not in phases:
                zz = pp.tile([P, FO], F32)
                nc.vector.memset(zz[:], 0.0)
                for t in range(T):
                    nc.sync.dma_start(out=out_d.ap()[t * P:(t + 1) * P, :], in_=zz[:])

            from bass_rust import add_dep_helper as _adh

            def add_dep_helper(a, b, reason=""):
                if isinstance(a, bass.BassInstruction):
                    a = a.ins
                if isinstance(b, bass.BassInstruction):
                    b = b.ins
                _adh(a, b, reason=reason)

            def spread_half(cat, tab, ag_inst, h, tag):
                """One tile-range half of the AllGather output -> padded
                256B-stride table. Returns per-segment spread-write fences."""
                agt = ag_pool.tile([P, N_CORES * CH * H], BF16, tag=f"agt{tag}{h}")
                ld = nc.sync.dma_start(
                    out=agt[:].rearrange("p (k c f) -> p k c f", k=N_CORES, c=CH),
                    in_=cat.ap().rearrange("(k p c) f -> p k c f", k=N_CORES, p=P),
                )
                add_dep_helper(ld, ag_inst, reason="allgather -> compact load")
                fences = {0: [], 1: []}
                for k in range(N_CORES):
                    spt = sp_tiles[(k + h * N_CORES) % n_sbufs]
                    nc.vector.tensor_copy(
                        out=spt[:].rearrange("p (c w) -> p c w", w=RP)[:, :, :H],
                        in_=agt[:].rearrange("p (k c f) -> p k c f", k=N_CORES, c=CH)[:, k, :, :],
                    )
                    # rows k*PC + p*T + (h*CH + c): per partition contiguous
                    eng = nc.sync if k % 2 else nc.scalar
                    wr = eng.dma_start(
                        out=tab.ap().rearrange("(k p c) w -> p k c w", k=N_CORES, p=P)
                            [:, k, h * CH:(h + 1) * CH, :],
                        in_=spt[:].rearrange("p (c w) -> p c w", w=RP),
                    )
                    fences[0 if k < N_CORES // 2 else 1].append(wr)
                return fences

            def ag_spread(locH, catH, tab, cwh, tag):
                fences = {0: [], 1: []}
                for h in range(2):
                    ag = nc.gpsimd.collective_compute(
                        "AllGather", mybir.AluOpType.bypass,
                        ins=[locH[h].ap()], outs=[catH[h].ap()], replica_groups=rg,
                    )
                    add_dep_helper(ag, cwh[h], reason="compact write -> allgather")
                    f = spread_half(catH[h], tab, ag, h, tag)
                    fences[0] += f[0]
                    fences[1] += f[1]
                return fences

            # ---- all-gather table1 (compact) + spread ----
            fences1 = None
            if "g" not in _os.environ.get("SKIP", "") and 2 in phases:
                fences1 = ag_spread(tab1_locH, tab1_catH, tab1, cw1h, 1)

            chunks_of = {}   # t -> number of chunks expected
            for g, runs, ow in schedule:
                for (t, l_lo, l_hi) in runs:
                    chunks_of[t] = chunks_of.get(t, 0) + (l_hi - l_lo)

            def agg_layer(tab_full, layer, epilogue, fences=None):
                """Packed calls; per-tile strided reduces; epilogue(t, u) fires
                once a tile's chunks are all reduced."""
                own = tab1_s if layer == 1 else tab2_s
                parts = {t: [] for t in range(T)}
                done = {t: 0 for t in range(T)}

                def finish(t):
                    pl = parts[t]
                    u = rp.tile([P, H], F32, tag=f"u{layer}")
                    nc.vector.tensor_add(out=u[:], in0=pl[0][:], in1=pl[1][:] if len(pl) > 1 else own[:, t * H:(t + 1) * H])
                    for extra in pl[2:]:
                        nc.vector.tensor_add(out=u[:], in0=u[:], in1=extra[:])
                    if len(pl) > 1:
                        nc.vector.tensor_add(out=u[:], in0=u[:], in1=own[:, t * H:(t + 1) * H])
                    epilogue(t, u)

                for call_i, (g, runs, ow) in enumerate(schedule):
                    C = sum(l_hi - l_lo for (_, l_lo, l_hi) in runs)
                    ni = C * P
                    gt = wp.tile([P, C * H], BF16, tag="g")
                    anchor = cfg.seg_anchor[g]
                    gi = DMA_GATHER_RAW(
                        nc.gpsimd,
                        gt[:].rearrange("p (c h) -> p c h", c=C),
                        tab_full.ap()[anchor:, :H],
                        idx_all[:, ow:ow + C * P // 16],
                        ni, ni, H, elem_step=RP,
                        queue_num=call_i % 4,
                        single_packet=False,
                    )
                    for f in (fences[g] if fences else ()):
                        add_dep_helper(gi, f, reason="table spread -> gather")
                    off = 0
                    for (t, l_lo, l_hi) in runs:
                        n = l_hi - l_lo
                        red = rp.tile([P, H], F32, tag=f"red{layer}")
                        nc.vector.tensor_reduce(
                            out=red[:],
                            in_=gt[:, off * H:(off + n) * H].rearrange("p (l h) -> p h l", h=H),
                            op=mybir.AluOpType.add, axis=mybir.AxisListType.X,
                        )
                        parts[t].append(red)
                        done[t] += n
                        off += n
                        if done[t] == chunks_of[t]:
                            finish(t)

            # ---- phase 2: layer-1 aggregation + epilogue -> table2 ----
            def epi1(t, u):
                # v = u*dinv + b1  -> table2_tile = dinv * relu(v)
                nc.vector.tensor_scalar_mul(out=u[:], in0=u[:], scalar1=dinv_t[:, t:t + 1])
                nc.vector.tensor_add(out=u[:], in0=u[:], in1=b1_t[:])
                nc.scalar.activation(
                    out=tab2_s[:, t * H:(t + 1) * H], in_=u[:],
                    func=mybir.ActivationFunctionType.Relu,
                    scale=dinv_t[:, t:t + 1],
                )
                if t == T - 1:
                    nc.vector.tensor_scalar_mul(
                        out=tab2_s[:, t * H:(t + 1) * H],
                        in0=tab2_s[:, t * H:(t + 1) * H], scalar1=mask_t[:, :1],
                    )

            if 2 in phases:
                agg_layer(tab1, 1, epi1, fences1)
                cw2h = [
                    nc.sync.dma_start(
                        out=tab2_locH[h].ap().rearrange("(p c) f -> p c f", p=P),
                        in_=tab2_s[:, h * CH * H:(h + 1) * CH * H].rearrange("p (c f) -> p c f", f=H),
                    )
                    for h in range(2)
                ]

            # ---- all-gather table2 (compact, two halves) + spread ----
            fences2 = None
            if 2 in phases and 3 in phases:
                fences2 = ag_spread(tab2_locH, tab2_catH, tab2, cw2h, 2)

            # ---- phase 3: layer-2 aggregation + W2 + b2 ----
            def epi2(t, u):
                # transpose u -> [H, P], matmul with W2, scale by dinv, + b2
                ps_t = psp.tile([P, P], F32, space="PSUM", tag="pst")
                nc.tensor.transpose(out=ps_t[:H, :], in_=u[:], identity=ident[:])
                uT = rp.tile([H, P], F32, tag="uT")
                nc.vector.tensor_copy(out=uT[:], in_=ps_t[:H, :])
                ps_o = psp.tile([P, FO], F32, space="PSUM", tag="pso")
                nc.tensor.matmul(out=ps_o[:], lhsT=uT[:], rhs=w2_t[:], start=True, stop=True)
                ot = rp.tile([P, FO], F32, tag="ot")
                nc.vector.tensor_scalar_mul(out=ot[:], in0=ps_o[:], scalar1=dinv_t[:, t:t + 1])
                nc.vector.tensor_add(out=ot[:], in0=ot[:], in1=b2_t[:])
                nc.sync.dma_start(out=out_d.ap()[t * P:(t + 1) * P, :], in_=ot[:])

            if 3 in phases and 2 in phases:
                agg_layer(tab2, 2, epi2, fences2)
            elif 2 in phases:
                zz = pp.tile([P, FO], F32)
                nc.vector.memset(zz[:], 0.0)
                for t in range(T):
                    nc.sync.dma_start(out=out_d.ap()[t * P:(t + 1) * P, :], in_=zz[:])

    nc.finalize()
    return nc


_CACHE = {}


def _get_compiled(cfg_key, cfg, prep):
    if cfg_key not in _CACHE:
        nc = build_bass(cfg, prep["schedule"], prep["total_w"])
        _CACHE[cfg_key] = nc
    return _CACHE[cfg_key]


def run(x, edge_index, W1, b1, W2, b2, cfg: Config, prep=None, nc=None, time_iters=0):
    x = np.asarray(x, dtype=np.float32)
    W1 = np.asarray(W1, dtype=np.float32)
    b1 = np.asarray(b1, dtype=np.float32)
    W2 = np.asarray(W2, dtype=np.float32)
    b2 = np.asarray(b2, dtype=np.float32)
    if prep is None:
        prep = host_prep(x, edge_index, cfg)
    if nc is None:
        nc = _get_compiled(("main", cfg.n_nodes, cfg.percore), cfg, prep)

    in_maps = build_in_maps(x, W1, b1, W2, b2, cfg, prep)
    res = run_bass_kernel_spmd(nc, in_maps, core_ids=list(range(N_CORES)))
    out_rows = np.concatenate([res.results[c]["out"] for c in range(N_CORES)], axis=0)
    out = out_rows[prep["row_of"]]
    return out


def build_in_maps(x, W1, b1, W2, b2, cfg: Config, prep):
    import ml_dtypes
    row_of = prep["row_of"]
    deg_full = prep["deg_full"].astype(np.float32)
    x_rows = np.zeros((cfg.nrows, cfg.f_in), dtype=np.float32)
    x_rows[row_of] = x
    deg_rows = np.ones(cfg.nrows, dtype=np.float32)
    deg_rows[row_of] = deg_full
    b1t = np.tile(np.asarray(b1)[None, :], (P, 1)).astype(np.float32)
    b2t = np.tile(np.asarray(b2)[None, :], (P, 1)).astype(np.float32)
    real_pc = cfg.n_nodes // N_CORES
    mask_last = np.zeros((P, 1), dtype=np.float32)
    lastt = cfg.tiles - 1
    for p in range(P):
        mask_last[p, 0] = 1.0 if lastt * P + p < real_pc else 0.0
    in_maps = []
    for c in range(N_CORES):
        xs = x_rows[c * cfg.percore:(c + 1) * cfg.percore]
        in_maps.append({
            "xt": np.ascontiguousarray(xs.T).astype(ml_dtypes.bfloat16),
            "w1": np.asarray(W1, np.float32).astype(ml_dtypes.bfloat16),
            "w2": np.asarray(W2, np.float32),
            "b1t": b1t, "b2t": b2t,
            "degt": deg_rows[c * cfg.percore:(c + 1) * cfg.percore],
            "maskt": mask_last,
            "idxs": prep["idx_arrays"][c],
        })
    return in_maps


def kernel(x, edge_index, W1, b1, W2, b2):
    cfg = Config(100000, 512, 16, 40, percore=12544)
    return run(x, edge_index, W1, b1, W2, b2, cfg)
